# revision 23
# baseline (speedup 1.0000x reference)
"""Trainium2 Bass kernel for nn_MultiHeadAttention (B=2, S=4096, F=512, H=8, causal).

Sharding: 8 cores = 2 (batch) x 4 (head pairs); host pre-transposes q/k/v per
batch, slices weights per head pair, sums the 4 bf16 partial outputs per batch
(replacing the all-reduce) and folds bv/bo as bv @ wo + bo (exact since
softmax rows sum to 1).

Engine plan per core (all matmul inputs bf16, attention weights bf16):
- Q/K/V projections in bf16 (host-cast inputs; FWL-fast weight loads);
  Q additionally pre-scaled by A_Q so logits land in log2-bits scale.
- Logits K^T Q per head with 64-contract stationaries at base partitions
  0/64 -> row-group-packed concurrent matmul pairs.
- exp on ScalarE (bf16 out); an optional DVE Schraudolph path exists
  (DVE_NUM/DVE_DEN share) but measured slower on HW, so it is disabled.
- PV per k-tile in bf16 with a ones column riding in the stationary to
  accumulate softmax denominators (fp8 was tried and rejected: concentrated
  softmax makes fp8 quantization cost ~5-7e-2 output error).
- Normalization: reciprocal of the denominator row, DMA-shift to partition 0,
  gpsimd partition broadcast, one fused DVE multiply during PSUM evacuation.
- Output projection accumulates both heads into one PSUM bank; evacuation
  alternates ScalarE/DVE; output DMA'd as bf16 and summed on host in f32.

The mask input is classified on the host into full / partial / skipped
[128 x 128] chunks and the device program is built (and cached) from that
schedule, so any 0/1-style additive mask produces a correct program.
"""

import numpy as np
import ml_dtypes
from contextlib import ExitStack

import concourse.bass as bass
import concourse.tile as tile
from concourse import bacc, mybir
from concourse import bass2jax
from concourse import dve_ops as _dvo
from concourse.dve_spec import Spec, Bin, AluOp as SAluOp, C0, C1, C2, Src0, Src1, sq

F32 = mybir.dt.float32
BF16 = mybir.dt.bfloat16
I16 = mybir.dt.int16
AF = mybir.ActivationFunctionType
ALU = mybir.AluOpType

NP_BF16 = ml_dtypes.bfloat16

B = 2
S = 4096
NF = 512
NH = 8
D = 64
N_CORES = 8
SQ = 512          # query block width
SK = 128          # key tile height
N_QB = S // SQ    # 8
N_SKT = S // SK   # 32
N_ST = S // 128   # 32
SCALE = 1.0 / np.sqrt(np.float32(D))  # 0.125
# fraction of clean (unmasked) exp tiles routed to the DVE fast-exp path
DVE_NUM, DVE_DEN = 4, 9

# --- custom DVE corrected bit-trick exp ------------------------------------ #
# Input x = QK product with Q pre-scaled by A_Q, so x = log2(exp target)*128
# (bf16 "bits scale"). Chain: y = x-64; magic-add rounds y to the nearest
# multiple of 128 in the [2^30, 2^31) binade -> N = 128*floor(x/128);
# g = x-64-N = 128*(f-0.5); parabolic binade correction beta*f*(1-f) folded
# as (g*C2)^2 with C2 = sqrt(-beta)/128; exponent bias 16256+64+beta/4 rides
# in Src1 (a [128,1] const tile). Output port converts f32->int16; the int16
# view is the bf16 bit pattern of exp. rms err ~0.26% vs exp.
A_Q = float(0.125 * 128.0 / np.log(2.0))   # pre-scale folded into Q proj
SCALE_SC = float(np.log(2.0) / 128.0)      # ScalarE exp scale on scaled logits
_FE_BETA = -43.93
FE_C0 = -64.0
FE_C1 = float(3 * 2.0 ** 29)
FE_C2 = float(np.sqrt(-_FE_BETA) / 128.0)
FE_S1 = float(16256.0 + 64.0 + _FE_BETA / 4.0)


def _fexp_reference(in0, in1, s0, s1, imm2):
    x = np.asarray(in0, np.float32)
    y = (x + np.float32(s0)).astype(np.float32)
    t = (y + np.float32(s1)).astype(np.float32)
    t2 = (t - np.float32(s1)).astype(np.float32)
    g = (y - t2).astype(np.float32)
    gs = (g * np.float32(imm2)).astype(np.float32)
    sqv = (gs * gs).astype(np.float32)
    o1 = (y + sqv).astype(np.float32)
    out = o1 + np.asarray(in1, np.float32)
    return np.rint(out).astype(np.float32)


def _make_fexp_op():
    y = Bin(SAluOp.ADD, Src0, C0)
    t = Bin(SAluOp.ADD, y, C1)
    t2 = Bin(SAluOp.SUBTRACT, t, C1)
    g = Bin(SAluOp.SUBTRACT, y, t2)
    gs = Bin(SAluOp.MULTIPLY, g, C2)
    o1 = Bin(SAluOp.ADD, y, sq(gs))
    body = Bin(SAluOp.ADD, o1, Src1)
    spec = Spec(body=body, reference=_fexp_reference)
    op = _dvo.DveOp("FEXP_BITS_ANT", spec, subdim=False, uops_sha={})
    if op.name not in {o.name for o in _dvo.OPS}:
        _dvo.OPS.append(op)
        _dvo.CUSTOM_DVE_SPECS[op.name] = op.spec
        _dvo._SUB_OPCODE_FOR_NAME[op.name] = max(
            _dvo._SUB_OPCODE_FOR_NAME.values()) + 1
    # pin the sha for whatever DVE version this install lowers to
    for ver in ("v3", "v4"):
        try:
            op.compile(ver)
        except ValueError as e:
            msg = str(e)
            key = f'uops_sha["{ver}"]="'
            if key in msg:
                op.uops_sha[ver] = msg.split(key, 1)[1].split('"', 1)[0]
            else:
                raise
    return op


FEXP_OP = _make_fexp_op()

_CACHE: dict = {}


def _classify_mask(mask: np.ndarray):
    """mask: [S, S] additive-style (nonzero => disallowed).

    Returns (schedule, patterns):
      schedule[qb] = tuple of items (sk, qlo, chunks) with
        chunks = tuple of (chunk_idx, pat_idx) for partially-masked
        [SK x 128] chunks (chunk_idx in units of 128 cols within the block).
      patterns: np.ndarray [n_pat, SK, 128] of multiplicative 0/1 masks.
    """
    m = mask != 0  # True => masked out; [q, k] per the reference
    schedule = []
    patterns = []
    pat_index: dict = {}
    for qb in range(N_QB):
        items = []
        for sk in range(N_SKT):
            sub = m[qb * SQ:(qb + 1) * SQ, sk * SK:(sk + 1) * SK].T  # [k, q]
            if sub.all():
                continue
            chunks = []
            qlo = None
            for ck in range(SQ // 128):
                csub = sub[:, ck * 128:(ck + 1) * 128]
                if csub.all():
                    continue  # fully masked chunk: skip if leading, else zero-pat
                if qlo is None:
                    qlo = ck * 128
                if csub.any():
                    pat = (~csub).astype(np.float32)
                    key = pat.tobytes()
                    if key not in pat_index:
                        pat_index[key] = len(patterns)
                        patterns.append(pat)
                    chunks.append((ck, pat_index[key]))
            if qlo is None:
                continue
            # interior fully-masked chunks (after qlo) need a zero pattern
            for ck in range(qlo // 128, SQ // 128):
                csub = sub[:, ck * 128:(ck + 1) * 128]
                if csub.all():
                    zpat = np.zeros((SK, 128), np.float32)
                    key = zpat.tobytes()
                    if key not in pat_index:
                        pat_index[key] = len(patterns)
                        patterns.append(zpat)
                    chunks.append((ck, pat_index[key]))
            chunks.sort()
            items.append((sk, qlo, tuple(chunks)))
        schedule.append(tuple(items))
    pats = (np.stack(patterns) if patterns
            else np.ones((1, SK, 128), np.float32))
    return tuple(schedule), pats


def _plan_units(items):
    """Assign exp engines per item (True = DVE fast-exp); masked items stay
    on ScalarE."""
    units = []
    full_idx = 0
    for it in items:
        clean = (it[1] == 0 and not it[2])
        if clean:
            dve = (full_idx * DVE_NUM) % DVE_DEN < DVE_NUM
            full_idx += 1
        else:
            dve = False
        units.append((it, dve))
    return units


def _build_program(schedule, n_pat, reps=1):
    nc = bacc.Bacc("TRN2", target_bir_lowering=False, debug=False,
                   num_devices=N_CORES)

    qT = nc.dram_tensor("qT", [NF, S], BF16, kind="ExternalInput").ap()
    kT = nc.dram_tensor("kT", [NF, S], BF16, kind="ExternalInput").ap()
    vT = nc.dram_tensor("vT", [NF, S], BF16, kind="ExternalInput").ap()
    wq_d = nc.dram_tensor("wq", [NF, 128], BF16, kind="ExternalInput").ap()
    wk_d = nc.dram_tensor("wk", [NF, 128], BF16, kind="ExternalInput").ap()
    wv_d = nc.dram_tensor("wv", [NF, 128], BF16, kind="ExternalInput").ap()
    wo_d = nc.dram_tensor("wo", [64, 2, NF], BF16, kind="ExternalInput").ap()
    bq_d = nc.dram_tensor("bq", [128, 1], F32, kind="ExternalInput").ap()
    bk_d = nc.dram_tensor("bk", [128, 1], F32, kind="ExternalInput").ap()
    msk_d = nc.dram_tensor("msk", [SK, n_pat * 128], BF16,
                           kind="ExternalInput").ap()
    o_d = nc.dram_tensor("o", [S, NF], BF16, kind="ExternalOutput").ap()

    with tile.TileContext(nc) as tc, ExitStack() as octx:
        per = octx.enter_context(tc.tile_pool(name="persist", bufs=1))

        QhT = per.tile([128, S], BF16, tag="qh")      # [head dims (A|B), S]
        KhT = per.tile([128, S], BF16, tag="kh")
        # fp8 V with ones col: [k-part, head, sk-tile, 80(65 used)]
        Vaug = per.tile([128, 2, N_SKT, 66], BF16, tag="vaug")
        attnA = per.tile([64, S], BF16, tag="attnA")  # normalized, head A
        attnB = per.tile([64, S], BF16, tag="attnB")
        wq_sb = per.tile([128, 4, 128], BF16, tag="wq")
        wk_sb = per.tile([128, 4, 128], BF16, tag="wk")
        wv_sb = per.tile([128, 4, 128], BF16, tag="wv")
        wo_sb = per.tile([64, 2, NF], BF16, tag="wo")
        bq_sb = per.tile([128, 1], F32, tag="bq")
        bk_sb = per.tile([128, 1], F32, tag="bk")
        msk_sb = per.tile([SK, n_pat * 128], BF16, tag="msk")
        fec_sb = per.tile([128, 1], F32, tag="fec")

        nc.vector.memset(Vaug, 0.0)
        nc.vector.memset(fec_sb, FE_S1)
        nc.vector.memset(Vaug.rearrange("p h t c -> p (h t) c")[:, :, 64:65],
                         1.0)
        nc.sync.dma_start(wq_sb, wq_d.rearrange("(c p) m -> p c m", p=128))
        nc.sync.dma_start(wk_sb, wk_d.rearrange("(c p) m -> p c m", p=128))
        nc.sync.dma_start(wv_sb, wv_d.rearrange("(c p) m -> p c m", p=128))
        nc.sync.dma_start(bq_sb, bq_d)
        nc.sync.dma_start(bk_sb, bk_d)
        nc.sync.dma_start(msk_sb, msk_d)
        nc.sync.dma_start(wo_sb, wo_d)

        for _rep in range(reps):
            # PSUM banks: xproj 2x[128,512] = 2, lt 2x[128,1024] = 4, pv 2.
            with tc.tile_pool(name="xs", bufs=3) as xs, \
                 tc.tile_pool(name="psqk", bufs=2, space="PSUM") as psqk, \
                 tc.tile_pool(name="pp", bufs=3) as pp, \
                 tc.tile_pool(name="ltp", bufs=2, space="PSUM") as ltp, \
                 tc.tile_pool(name="pvp", bufs=2, space="PSUM") as pvp, \
                 tc.tile_pool(name="ev", bufs=2) as ev, \
                 tc.tile_pool(name="ob", bufs=3) as ob:

                def emit_proj(qb):
                    qsl = slice(qb * SQ, (qb + 1) * SQ)
                    for dst, src, w_s, b_s, qscale in (
                            (KhT, kT, wk_sb, bk_sb, None),
                            (QhT, qT, wq_sb, bq_sb, A_Q)):
                        pt = psqk.tile([128, SQ], F32, tag="qk")
                        xb = xs.tile([128, 4, SQ], BF16, tag="x", bufs=3)
                        nc.sync.dma_start(
                            xb, src.rearrange("(c p) m -> p c m", p=128)[:, :, qsl])
                        for f in range(4):
                            nc.tensor.matmul(pt, w_s[:, f, :], xb[:, f, :],
                                             start=(f == 0), stop=(f == 3))
                        if qscale is None:
                            nc.vector.tensor_scalar_add(dst[:, qsl], pt, b_s)
                        else:
                            nc.vector.tensor_scalar(dst[:, qsl], pt, b_s,
                                                    qscale, ALU.add, ALU.mult)
                    vbig = xs.tile([128, 4, SQ], BF16, tag="x")
                    nc.sync.dma_start(
                        vbig, vT.rearrange("(c p) m -> p c m", p=128)[:, :, qsl])
                    for j in range(4):
                        st = 4 * qb + j
                        pv_ = psqk.tile([128, SQ], F32, tag="qk")
                        for f in range(4):
                            nc.tensor.matmul(pv_[:, 0:128],
                                             vbig[:, f, j * 128:(j + 1) * 128],
                                             wv_sb[:, f, :],
                                             start=(f == 0), stop=(f == 3))
                        nc.vector.tensor_copy(
                            Vaug[:, :, st, 0:64],
                            pv_[:, 0:128].rearrange("p (h d) -> p h d", h=2))

                def emit_exp(lt, pab, qlo, dve):
                    """exp of lt[:, qlo:512 | 512+qlo:1024] -> pab"""
                    if qlo == 0:
                        osl = pab
                        isl = lt
                    else:
                        osl = pab.rearrange(
                            "p (two q) -> p two q", q=SQ)[:, :, qlo:SQ]
                        isl = lt.rearrange(
                            "p (two q) -> p two q", q=SQ)[:, :, qlo:SQ]
                    if dve:
                        nc.vector.tensor_scalar(osl.bitcast(I16), isl, 1.0,
                                                16250.4, ALU.mult, ALU.add)
                    else:
                        nc.scalar.activation(osl, isl, AF.Exp,
                                             bias=0.0, scale=SCALE_SC)

                def emit_attn(qb):
                    qsl = slice(qb * SQ, (qb + 1) * SQ)
                    q0 = qb * SQ
                    items = schedule[qb]
                    if not items:
                        return
                    units = _plan_units(items)
                    pvA = pvp.tile([65, SQ], F32, tag="pv")
                    pvB = pvp.tile([65, SQ], F32, tag="pv")
                    n_units = len(units)
                    for ui, ((sk, qlo, chunks), dve) in enumerate(units):
                        st_flag = (ui == 0)
                        sp_flag = (ui == n_units - 1)
                        pab = pp.tile([128, 1024], BF16, tag="pab")
                        ksl = slice(sk * SK, (sk + 1) * SK)
                        qs = slice(q0 + qlo, q0 + SQ)
                        lt = ltp.tile([128, 1024], F32, tag="lt")
                        nc.tensor.matmul(lt[:, qlo:SQ], KhT[0:64, ksl],
                                         QhT[0:64, qs],
                                         start=True, stop=True)
                        nc.tensor.matmul(lt[:, SQ + qlo:2 * SQ],
                                         KhT[64:128, ksl],
                                         QhT[64:128, qs],
                                         start=True, stop=True)
                        emit_exp(lt, pab, qlo, dve)
                        for ck, pat in chunks:
                            msl = msk_sb[:, pat * 128:(pat + 1) * 128]
                            for h in range(2):
                                psl = pab[:,
                                          h * SQ + ck * 128:
                                          h * SQ + (ck + 1) * 128]
                                nc.vector.tensor_tensor(psl, psl, msl,
                                                        ALU.mult)
                        nc.tensor.matmul(pvA[:, qlo:SQ],
                                         Vaug[:, 0, sk, 0:65],
                                         pab[:, qlo:SQ],
                                         start=st_flag, stop=sp_flag)
                        nc.tensor.matmul(pvB[:, qlo:SQ],
                                         Vaug[:, 1, sk, 0:65],
                                         pab[:, SQ + qlo:2 * SQ],
                                         start=st_flag, stop=sp_flag)
                    # evacuate PSUM fast (frees pv banks for the next block),
                    # then normalize off the critical path: recip of the
                    # denominator row (lane 64), DMA-shift to partition 0
                    # (partition_broadcast reads physical partition 0 on HW),
                    # broadcast, fused multiply into bf16 attn
                    pvsA = ev.tile([65, SQ], F32, tag="pvs")
                    pvsB = ev.tile([65, SQ], F32, tag="pvs")
                    nc.vector.tensor_copy(pvsA, pvA)
                    nc.vector.tensor_copy(pvsB, pvB)
                    for pvs, attn in ((pvsA, attnA), (pvsB, attnB)):
                        # shift the denominator row to partition 0 first:
                        # approx recip (like partition_broadcast) reads the
                        # wrong partitions at non-zero base on HW
                        rr0 = ev.tile([1, SQ], F32, tag="rr0")
                        nc.scalar.dma_start(rr0, pvs[64:65, :])
                        rrec = ev.tile([1, SQ], F32, tag="rrec")
                        nc.vector.reciprocal_approx_fast(rrec, rr0)
                        bc = ev.tile([64, SQ], F32, tag="bc")
                        nc.gpsimd.partition_broadcast(bc, rrec)
                        nc.vector.tensor_tensor(attn[:, qsl], pvs[0:64, :], bc,
                                                ALU.mult)

                def emit_outproj(qb):
                    for j in range(4):
                        st = 4 * qb + j
                        sl = slice(st * 128, (st + 1) * 128)
                        oo = psqk.tile([128, SQ], F32, tag="qk")
                        nc.tensor.matmul(oo, attnA[:, sl], wo_sb[:, 0, :],
                                         start=True, stop=False)
                        nc.tensor.matmul(oo, attnB[:, sl], wo_sb[:, 1, :],
                                         start=False, stop=True)
                        osb = ob.tile([128, NF], BF16, tag="os")
                        nc.vector.tensor_copy(osb, oo)
                        nc.sync.dma_start(o_d[sl, :], osb)

                emit_proj(0)
                emit_proj(1)
                for qb in range(N_QB):
                    emit_attn(qb)
                    if qb + 2 < N_QB:
                        emit_proj(qb + 2)
                    emit_outproj(qb)

    nc.compile()
    return nc


def _prep_core_inputs(c, q, k, v, wq, bq, wk, bk, wv, patterns):
    b = c // 4
    hp = c % 4
    cols = slice(128 * hp, 128 * (hp + 1))
    n_pat = patterns.shape[0]
    wo_slice = _prep_core_inputs._wo[cols, :]  # [128, 512]
    return {
        "qT": np.ascontiguousarray(q[b].T).astype(NP_BF16),
        "kT": np.ascontiguousarray(k[b].T).astype(NP_BF16),
        "vT": np.ascontiguousarray(v[b].T).astype(NP_BF16),
        "wq": np.ascontiguousarray(wq[:, cols]).astype(NP_BF16),
        "wk": np.ascontiguousarray(wk[:, cols]).astype(NP_BF16),
        "wv": np.ascontiguousarray(wv[:, cols]).astype(NP_BF16),
        "wo": np.ascontiguousarray(
            wo_slice.reshape(2, 64, NF).transpose(1, 0, 2)).astype(NP_BF16),
        "bq": np.ascontiguousarray(bq[cols].reshape(128, 1)),
        "bk": np.ascontiguousarray(bk[cols].reshape(128, 1)),
        "msk": np.ascontiguousarray(
            patterns.transpose(1, 0, 2).reshape(SK, n_pat * 128)
        ).astype(NP_BF16),
    }


def get_state(mask_np, reps=1):
    """Build (or fetch cached) compiled program + schedule for this mask."""
    mask2d = np.asarray(mask_np, dtype=np.float32).reshape(S, S)
    schedule, patterns = _classify_mask(mask2d)
    key = (schedule, patterns.tobytes(), reps)
    if key not in _CACHE:
        nc = _build_program(schedule, patterns.shape[0], reps=reps)
        _CACHE[key] = {"nc": nc, "schedule": schedule, "patterns": patterns}
    return _CACHE[key]


def kernel(q, k, v, mask, wq, bq, wk, bk, wv, bv, wo, bo):
    q = np.asarray(q, np.float32)
    k = np.asarray(k, np.float32)
    v = np.asarray(v, np.float32)
    wq_n = np.asarray(wq, np.float32)
    wk_n = np.asarray(wk, np.float32)
    wv_n = np.asarray(wv, np.float32)
    wo_n = np.asarray(wo, np.float32)
    bq_n = np.asarray(bq, np.float32)
    bk_n = np.asarray(bk, np.float32)
    bv_n = np.asarray(bv, np.float32)
    bo_n = np.asarray(bo, np.float32)

    state = get_state(mask)
    nc = state["nc"]
    patterns = state["patterns"]

    _prep_core_inputs._wo = wo_n
    in_maps = [
        _prep_core_inputs(c, q, k, v, wq_n, bq_n, wk_n, bk_n, wv_n, patterns)
        for c in range(N_CORES)
    ]
    results = bass2jax.run_bass_via_pjrt(nc, in_maps, n_cores=N_CORES)

    bo_eff = bv_n @ wo_n + bo_n  # exact: softmax rows sum to 1
    out = np.empty((B, S, NF), np.float32)
    for b in range(B):
        acc = results[b * 4 + 0]["o"].astype(np.float32)
        for hp in range(1, 4):
            acc = acc + results[b * 4 + hp]["o"].astype(np.float32)
        out[b] = acc + bo_eff
    return out


# revision 24
# speedup vs baseline: 1.0836x; 1.0836x over previous
"""Trainium2 Bass kernel for nn_MultiHeadAttention (B=2, S=4096, F=512, H=8, causal).

Sharding: 8 cores = 2 (batch) x 4 (head pairs); host pre-transposes q/k/v per
batch, slices weights per head pair, sums the 4 bf16 partial outputs per batch
(replacing the all-reduce) and folds bv/bo as bv @ wo + bo (exact since
softmax rows sum to 1).

Engine plan per core (all matmul inputs bf16, attention weights bf16):
- Q/K/V projections in bf16 (host-cast inputs; FWL-fast weight loads);
  Q additionally pre-scaled by A_Q so logits land in log2-bits scale.
- Logits K^T Q per head with 64-contract stationaries at base partitions
  0/64 -> row-group-packed concurrent matmul pairs.
- exp on ScalarE (bf16 out); an optional DVE Schraudolph path exists
  (DVE_NUM/DVE_DEN share) but measured slower on HW, so it is disabled.
- PV per k-tile in bf16 with a ones column riding in the stationary to
  accumulate softmax denominators (fp8 was tried and rejected: concentrated
  softmax makes fp8 quantization cost ~5-7e-2 output error).
- Normalization: reciprocal of the denominator row, DMA-shift to partition 0,
  gpsimd partition broadcast, one fused DVE multiply during PSUM evacuation.
- Output projection accumulates both heads into one PSUM bank; evacuation
  alternates ScalarE/DVE; output DMA'd as bf16 and summed on host in f32.

The mask input is classified on the host into full / partial / skipped
[128 x 128] chunks and the device program is built (and cached) from that
schedule, so any 0/1-style additive mask produces a correct program.
"""

import numpy as np
import ml_dtypes
from contextlib import ExitStack

import concourse.bass as bass
import concourse.tile as tile
from concourse import bacc, mybir
from concourse import bass2jax
from concourse import dve_ops as _dvo
from concourse.dve_spec import Spec, Bin, AluOp as SAluOp, C0, C1, C2, Src0, Src1, sq

F32 = mybir.dt.float32
BF16 = mybir.dt.bfloat16
I16 = mybir.dt.int16
AF = mybir.ActivationFunctionType
ALU = mybir.AluOpType

NP_BF16 = ml_dtypes.bfloat16

B = 2
S = 4096
NF = 512
NH = 8
D = 64
N_CORES = 8
SQ = 512          # query block width
SK = 128          # key tile height
N_QB = S // SQ    # 8
N_SKT = S // SK   # 32
N_ST = S // 128   # 32
SCALE = 1.0 / np.sqrt(np.float32(D))  # 0.125
# fraction of clean (unmasked) exp tiles routed to the DVE fast-exp path
DVE_NUM, DVE_DEN = 4, 9

# --- custom DVE corrected bit-trick exp ------------------------------------ #
# Input x = QK product with Q pre-scaled by A_Q, so x = log2(exp target)*128
# (bf16 "bits scale"). Chain: y = x-64; magic-add rounds y to the nearest
# multiple of 128 in the [2^30, 2^31) binade -> N = 128*floor(x/128);
# g = x-64-N = 128*(f-0.5); parabolic binade correction beta*f*(1-f) folded
# as (g*C2)^2 with C2 = sqrt(-beta)/128; exponent bias 16256+64+beta/4 rides
# in Src1 (a [128,1] const tile). Output port converts f32->int16; the int16
# view is the bf16 bit pattern of exp. rms err ~0.26% vs exp.
A_Q = float(0.125 * 128.0 / np.log(2.0))   # pre-scale folded into Q proj
SCALE_SC = float(np.log(2.0) / 128.0)      # ScalarE exp scale on scaled logits
_FE_BETA = -43.93
FE_C0 = -64.0
FE_C1 = float(3 * 2.0 ** 29)
FE_C2 = float(np.sqrt(-_FE_BETA) / 128.0)
FE_S1 = float(16256.0 + 64.0 + _FE_BETA / 4.0)


def _fexp_reference(in0, in1, s0, s1, imm2):
    x = np.asarray(in0, np.float32)
    y = (x + np.float32(s0)).astype(np.float32)
    t = (y + np.float32(s1)).astype(np.float32)
    t2 = (t - np.float32(s1)).astype(np.float32)
    g = (y - t2).astype(np.float32)
    gs = (g * np.float32(imm2)).astype(np.float32)
    sqv = (gs * gs).astype(np.float32)
    o1 = (y + sqv).astype(np.float32)
    out = o1 + np.asarray(in1, np.float32)
    return np.rint(out).astype(np.float32)


def _make_fexp_op():
    y = Bin(SAluOp.ADD, Src0, C0)
    t = Bin(SAluOp.ADD, y, C1)
    t2 = Bin(SAluOp.SUBTRACT, t, C1)
    g = Bin(SAluOp.SUBTRACT, y, t2)
    gs = Bin(SAluOp.MULTIPLY, g, C2)
    o1 = Bin(SAluOp.ADD, y, sq(gs))
    body = Bin(SAluOp.ADD, o1, Src1)
    spec = Spec(body=body, reference=_fexp_reference)
    op = _dvo.DveOp("FEXP_BITS_ANT", spec, subdim=False, uops_sha={})
    if op.name not in {o.name for o in _dvo.OPS}:
        _dvo.OPS.append(op)
        _dvo.CUSTOM_DVE_SPECS[op.name] = op.spec
        _dvo._SUB_OPCODE_FOR_NAME[op.name] = max(
            _dvo._SUB_OPCODE_FOR_NAME.values()) + 1
    # pin the sha for whatever DVE version this install lowers to
    for ver in ("v3", "v4"):
        try:
            op.compile(ver)
        except ValueError as e:
            msg = str(e)
            key = f'uops_sha["{ver}"]="'
            if key in msg:
                op.uops_sha[ver] = msg.split(key, 1)[1].split('"', 1)[0]
            else:
                raise
    return op


FEXP_OP = _make_fexp_op()

_CACHE: dict = {}


def _classify_mask(mask: np.ndarray):
    """mask: [S, S] additive-style (nonzero => disallowed).

    Returns (schedule, patterns):
      schedule[qb] = tuple of items (sk, qlo, chunks) with
        chunks = tuple of (chunk_idx, pat_idx) for partially-masked
        [SK x 128] chunks (chunk_idx in units of 128 cols within the block).
      patterns: np.ndarray [n_pat, SK, 128] of multiplicative 0/1 masks.
    """
    m = mask != 0  # True => masked out; [q, k] per the reference
    schedule = []
    patterns = []
    pat_index: dict = {}
    for qb in range(N_QB):
        items = []
        for sk in range(N_SKT):
            sub = m[qb * SQ:(qb + 1) * SQ, sk * SK:(sk + 1) * SK].T  # [k, q]
            if sub.all():
                continue
            chunks = []
            qlo = None
            for ck in range(SQ // 128):
                csub = sub[:, ck * 128:(ck + 1) * 128]
                if csub.all():
                    continue  # fully masked chunk: skip if leading, else zero-pat
                if qlo is None:
                    qlo = ck * 128
                if csub.any():
                    pat = (~csub).astype(np.float32)
                    key = pat.tobytes()
                    if key not in pat_index:
                        pat_index[key] = len(patterns)
                        patterns.append(pat)
                    chunks.append((ck, pat_index[key]))
            if qlo is None:
                continue
            # interior fully-masked chunks (after qlo) need a zero pattern
            for ck in range(qlo // 128, SQ // 128):
                csub = sub[:, ck * 128:(ck + 1) * 128]
                if csub.all():
                    zpat = np.zeros((SK, 128), np.float32)
                    key = zpat.tobytes()
                    if key not in pat_index:
                        pat_index[key] = len(patterns)
                        patterns.append(zpat)
                    chunks.append((ck, pat_index[key]))
            chunks.sort()
            items.append((sk, qlo, tuple(chunks)))
        schedule.append(tuple(items))
    pats = (np.stack(patterns) if patterns
            else np.ones((1, SK, 128), np.float32))
    return tuple(schedule), pats


def _plan_units(items):
    """Assign exp engines per item (True = DVE fast-exp); masked items stay
    on ScalarE."""
    units = []
    full_idx = 0
    for it in items:
        clean = (it[1] == 0 and not it[2])
        if clean:
            dve = (full_idx * DVE_NUM) % DVE_DEN < DVE_NUM
            full_idx += 1
        else:
            dve = False
        units.append((it, dve))
    return units


def _build_program(schedule, n_pat, reps=1):
    nc = bacc.Bacc("TRN2", target_bir_lowering=False, debug=False,
                   num_devices=N_CORES)

    qT = nc.dram_tensor("qT", [NF, S], BF16, kind="ExternalInput").ap()
    kT = nc.dram_tensor("kT", [NF, S], BF16, kind="ExternalInput").ap()
    vT = nc.dram_tensor("vT", [NF, S], BF16, kind="ExternalInput").ap()
    wq_d = nc.dram_tensor("wq", [NF, 128], BF16, kind="ExternalInput").ap()
    wk_d = nc.dram_tensor("wk", [NF, 128], BF16, kind="ExternalInput").ap()
    wv_d = nc.dram_tensor("wv", [NF, 128], BF16, kind="ExternalInput").ap()
    wo_d = nc.dram_tensor("wo", [64, 2, NF], BF16, kind="ExternalInput").ap()
    bq_d = nc.dram_tensor("bq", [128, 1], F32, kind="ExternalInput").ap()
    bk_d = nc.dram_tensor("bk", [128, 1], F32, kind="ExternalInput").ap()
    msk_d = nc.dram_tensor("msk", [SK, n_pat * 128], BF16,
                           kind="ExternalInput").ap()
    o_d = nc.dram_tensor("o", [S, NF], BF16, kind="ExternalOutput").ap()

    with tile.TileContext(nc) as tc, ExitStack() as octx:
        per = octx.enter_context(tc.tile_pool(name="persist", bufs=1))

        QhT = per.tile([128, S], BF16, tag="qh")      # [head dims (A|B), S]
        KhT = per.tile([128, S], BF16, tag="kh")
        # fp8 V with ones col: [k-part, head, sk-tile, 80(65 used)]
        Vaug = per.tile([128, 2, N_SKT, 66], BF16, tag="vaug")
        attnA = per.tile([64, S], BF16, tag="attnA")  # normalized, head A
        attnB = per.tile([64, S], BF16, tag="attnB")
        wq_sb = per.tile([128, 4, 128], BF16, tag="wq")
        wk_sb = per.tile([128, 4, 128], BF16, tag="wk")
        wv_sb = per.tile([128, 4, 128], BF16, tag="wv")
        wo_sb = per.tile([64, 2, NF], BF16, tag="wo")
        bq_sb = per.tile([128, 1], F32, tag="bq")
        bk_sb = per.tile([128, 1], F32, tag="bk")
        msk_sb = per.tile([SK, n_pat * 128], BF16, tag="msk")
        fec_sb = per.tile([128, 1], F32, tag="fec")

        nc.vector.memset(Vaug, 0.0)
        nc.vector.memset(fec_sb, FE_S1)
        nc.vector.memset(Vaug.rearrange("p h t c -> p (h t) c")[:, :, 64:65],
                         1.0)
        nc.sync.dma_start(wq_sb, wq_d.rearrange("(c p) m -> p c m", p=128))
        nc.sync.dma_start(wk_sb, wk_d.rearrange("(c p) m -> p c m", p=128))
        nc.sync.dma_start(wv_sb, wv_d.rearrange("(c p) m -> p c m", p=128))
        nc.sync.dma_start(bq_sb, bq_d)
        nc.sync.dma_start(bk_sb, bk_d)
        nc.sync.dma_start(msk_sb, msk_d)
        nc.sync.dma_start(wo_sb, wo_d)

        for _rep in range(reps):
            # PSUM banks: xproj 2x[128,512] = 2, lt 2x[128,1024] = 4, pv 2.
            with tc.tile_pool(name="xs", bufs=3) as xs, \
                 tc.tile_pool(name="psqk", bufs=2, space="PSUM") as psqk, \
                 tc.tile_pool(name="pp", bufs=3) as pp, \
                 tc.tile_pool(name="ltp", bufs=2, space="PSUM") as ltp, \
                 tc.tile_pool(name="pvp", bufs=2, space="PSUM") as pvp, \
                 tc.tile_pool(name="ev", bufs=2) as ev, \
                 tc.tile_pool(name="ob", bufs=3) as ob:

                def emit_proj(qb):
                    qsl = slice(qb * SQ, (qb + 1) * SQ)
                    for dst, src, w_s, b_s, qscale in (
                            (KhT, kT, wk_sb, bk_sb, None),
                            (QhT, qT, wq_sb, bq_sb, A_Q)):
                        pt = psqk.tile([128, SQ], F32, tag="qk")
                        xb = xs.tile([128, 4, SQ], BF16, tag="x", bufs=3)
                        nc.sync.dma_start(
                            xb, src.rearrange("(c p) m -> p c m", p=128)[:, :, qsl])
                        for f in range(4):
                            nc.tensor.matmul(pt, w_s[:, f, :], xb[:, f, :],
                                             start=(f == 0), stop=(f == 3))
                        if qscale is None:
                            nc.vector.tensor_scalar_add(dst[:, qsl], pt, b_s)
                        else:
                            nc.vector.tensor_scalar(dst[:, qsl], pt, b_s,
                                                    qscale, ALU.add, ALU.mult)
                    vbig = xs.tile([128, 4, SQ], BF16, tag="x")
                    nc.sync.dma_start(
                        vbig, vT.rearrange("(c p) m -> p c m", p=128)[:, :, qsl])
                    for j in range(4):
                        st = 4 * qb + j
                        pv_ = psqk.tile([128, SQ], F32, tag="qk")
                        for f in range(4):
                            nc.tensor.matmul(pv_[:, 0:128],
                                             vbig[:, f, j * 128:(j + 1) * 128],
                                             wv_sb[:, f, :],
                                             start=(f == 0), stop=(f == 3))
                        nc.vector.tensor_copy(
                            Vaug[:, :, st, 0:64],
                            pv_[:, 0:128].rearrange("p (h d) -> p h d", h=2))

                def emit_exp(lt, pab, qlo, dve):
                    """exp of lt[:, qlo:512 | 512+qlo:1024] -> pab"""
                    if qlo == 0:
                        osl = pab
                        isl = lt
                    else:
                        osl = pab.rearrange(
                            "p (two q) -> p two q", q=SQ)[:, :, qlo:SQ]
                        isl = lt.rearrange(
                            "p (two q) -> p two q", q=SQ)[:, :, qlo:SQ]
                    if dve:
                        nc.vector.tensor_scalar(osl.bitcast(I16), isl, 1.0,
                                                16250.4, ALU.mult, ALU.add)
                    else:
                        nc.scalar.activation(osl, isl, AF.Exp,
                                             bias=0.0, scale=SCALE_SC)

                def emit_attn(qb):
                    qsl = slice(qb * SQ, (qb + 1) * SQ)
                    q0 = qb * SQ
                    items = schedule[qb]
                    if not items:
                        return
                    units = _plan_units(items)
                    pvA = pvp.tile([65, SQ], F32, tag="pv")
                    pvB = pvp.tile([65, SQ], F32, tag="pv")
                    n_units = len(units)
                    for ui, ((sk, qlo, chunks), dve) in enumerate(units):
                        st_flag = (ui == 0)
                        sp_flag = (ui == n_units - 1)
                        pab = pp.tile([128, 1024], BF16, tag="pab")
                        ksl = slice(sk * SK, (sk + 1) * SK)
                        qs = slice(q0 + qlo, q0 + SQ)
                        lt = ltp.tile([128, 1024], F32, tag="lt")
                        nc.tensor.matmul(lt[:, qlo:SQ], KhT[0:64, ksl],
                                         QhT[0:64, qs],
                                         start=True, stop=True)
                        nc.tensor.matmul(lt[:, SQ + qlo:2 * SQ],
                                         KhT[64:128, ksl],
                                         QhT[64:128, qs],
                                         start=True, stop=True)
                        emit_exp(lt, pab, qlo, dve)
                        for ck, pat in chunks:
                            msl = msk_sb[:, pat * 128:(pat + 1) * 128]
                            for h in range(2):
                                psl = pab[:,
                                          h * SQ + ck * 128:
                                          h * SQ + (ck + 1) * 128]
                                nc.vector.tensor_tensor(psl, psl, msl,
                                                        ALU.mult)
                        nc.tensor.matmul(pvA[:, qlo:SQ],
                                         Vaug[:, 0, sk, 0:65],
                                         pab[:, qlo:SQ],
                                         start=st_flag, stop=sp_flag)
                        nc.tensor.matmul(pvB[:, qlo:SQ],
                                         Vaug[:, 1, sk, 0:65],
                                         pab[:, SQ + qlo:2 * SQ],
                                         start=st_flag, stop=sp_flag)
                    # evacuate PSUM fast (frees pv banks for the next block),
                    # then normalize off the critical path: recip of the
                    # denominator row (lane 64), DMA-shift to partition 0
                    # (partition_broadcast reads physical partition 0 on HW),
                    # broadcast, fused multiply into bf16 attn
                    pvsA = ev.tile([65, SQ], F32, tag="pvs")
                    pvsB = ev.tile([65, SQ], F32, tag="pvs")
                    nc.scalar.activation(pvsA, pvA, AF.Copy)
                    nc.vector.tensor_copy(pvsB, pvB)
                    for pvs, attn in ((pvsA, attnA), (pvsB, attnB)):
                        # shift the denominator row to partition 0 first:
                        # approx recip (like partition_broadcast) reads the
                        # wrong partitions at non-zero base on HW
                        rr0 = ev.tile([1, SQ], F32, tag="rr0")
                        nc.scalar.dma_start(rr0, pvs[64:65, :])
                        rrec = ev.tile([1, SQ], F32, tag="rrec")
                        nc.vector.reciprocal_approx_fast(rrec, rr0)
                        bc = ev.tile([64, SQ], F32, tag="bc")
                        nc.gpsimd.partition_broadcast(bc, rrec)
                        nc.vector.tensor_tensor(attn[:, qsl], pvs[0:64, :], bc,
                                                ALU.mult)

                def emit_outproj(qb):
                    for j in range(4):
                        st = 4 * qb + j
                        sl = slice(st * 128, (st + 1) * 128)
                        oo = psqk.tile([128, SQ], F32, tag="qk")
                        nc.tensor.matmul(oo, attnA[:, sl], wo_sb[:, 0, :],
                                         start=True, stop=False)
                        nc.tensor.matmul(oo, attnB[:, sl], wo_sb[:, 1, :],
                                         start=False, stop=True)
                        osb = ob.tile([128, NF], BF16, tag="os")
                        if j % 2 == 0:
                            nc.scalar.activation(osb, oo, AF.Copy)
                        else:
                            nc.vector.tensor_copy(osb, oo)
                        nc.sync.dma_start(o_d[sl, :], osb)

                emit_proj(0)
                emit_proj(1)
                for qb in range(N_QB):
                    emit_attn(qb)
                    if qb + 2 < N_QB:
                        emit_proj(qb + 2)
                    emit_outproj(qb)

    nc.compile()
    return nc


def _prep_core_inputs(c, q, k, v, wq, bq, wk, bk, wv, patterns):
    b = c // 4
    hp = c % 4
    cols = slice(128 * hp, 128 * (hp + 1))
    n_pat = patterns.shape[0]
    wo_slice = _prep_core_inputs._wo[cols, :]  # [128, 512]
    return {
        "qT": np.ascontiguousarray(q[b].T).astype(NP_BF16),
        "kT": np.ascontiguousarray(k[b].T).astype(NP_BF16),
        "vT": np.ascontiguousarray(v[b].T).astype(NP_BF16),
        "wq": np.ascontiguousarray(wq[:, cols]).astype(NP_BF16),
        "wk": np.ascontiguousarray(wk[:, cols]).astype(NP_BF16),
        "wv": np.ascontiguousarray(wv[:, cols]).astype(NP_BF16),
        "wo": np.ascontiguousarray(
            wo_slice.reshape(2, 64, NF).transpose(1, 0, 2)).astype(NP_BF16),
        "bq": np.ascontiguousarray(bq[cols].reshape(128, 1)),
        "bk": np.ascontiguousarray(bk[cols].reshape(128, 1)),
        "msk": np.ascontiguousarray(
            patterns.transpose(1, 0, 2).reshape(SK, n_pat * 128)
        ).astype(NP_BF16),
    }


def get_state(mask_np, reps=1):
    """Build (or fetch cached) compiled program + schedule for this mask."""
    mask2d = np.asarray(mask_np, dtype=np.float32).reshape(S, S)
    schedule, patterns = _classify_mask(mask2d)
    key = (schedule, patterns.tobytes(), reps)
    if key not in _CACHE:
        nc = _build_program(schedule, patterns.shape[0], reps=reps)
        _CACHE[key] = {"nc": nc, "schedule": schedule, "patterns": patterns}
    return _CACHE[key]


def kernel(q, k, v, mask, wq, bq, wk, bk, wv, bv, wo, bo):
    q = np.asarray(q, np.float32)
    k = np.asarray(k, np.float32)
    v = np.asarray(v, np.float32)
    wq_n = np.asarray(wq, np.float32)
    wk_n = np.asarray(wk, np.float32)
    wv_n = np.asarray(wv, np.float32)
    wo_n = np.asarray(wo, np.float32)
    bq_n = np.asarray(bq, np.float32)
    bk_n = np.asarray(bk, np.float32)
    bv_n = np.asarray(bv, np.float32)
    bo_n = np.asarray(bo, np.float32)

    state = get_state(mask)
    nc = state["nc"]
    patterns = state["patterns"]

    _prep_core_inputs._wo = wo_n
    in_maps = [
        _prep_core_inputs(c, q, k, v, wq_n, bq_n, wk_n, bk_n, wv_n, patterns)
        for c in range(N_CORES)
    ]
    results = bass2jax.run_bass_via_pjrt(nc, in_maps, n_cores=N_CORES)

    bo_eff = bv_n @ wo_n + bo_n  # exact: softmax rows sum to 1
    out = np.empty((B, S, NF), np.float32)
    for b in range(B):
        acc = results[b * 4 + 0]["o"].astype(np.float32)
        for hp in range(1, 4):
            acc = acc + results[b * 4 + hp]["o"].astype(np.float32)
        out[b] = acc + bo_eff
    return out
